# revision 13
# baseline (speedup 1.0000x reference)
"""ChannelRoll Trainium2 Bass kernel.

out[b,h,w,c] = x[b,h,w,(c + shift_map[b,h,w,0]) % 256]

Strategy (pure data-parallel over batch, 8 cores):
  Per core, output row r (of 12544 rows x 256 fp32) is the cyclic
  window of input row r starting at m_r.  The two segments of the
  rolled row live ADJACENTLY in a row-padded flat copy of x:

    x_pad_flat[256*r + m : +512] = [ .. | x[r, :m] | x[r, m:] | .. ]
        chunk[256-m_r : 256]  = x[r, :m_r]   (the wrap tail)
        chunk[256 : 512-m_r]  = x[r, m_r:]   (the head)

  One indirect DMA gather per 128 rows (one element-granular offset
  per partition - the hardware DGE's supported form) fetches the
  512-f32 chunk per row.  A single in-place DVE copy_predicated
  overlays chunk[256+j] onto chunk[j] for j < 256-m (mask from one
  cheap int16 iota compare); chunk[0:256] is then the rolled row and
  is stored with a strided HWDGE DMA.  No GPSIMD compute anywhere;
  the roll runs at DMA/SWDGE speed (memory roofline regime).

  Per-core traffic: 25.7 MB gathered read (2x of x - fixed-size
  chunks must cover both segments) + 12.8 MB write; 98 SWDGE gather
  instructions (~1 us each) overlap the SDMA drain; DVE does ~1.5
  passes, fully hidden.
"""

import numpy as np

B, H, W, C = 32, 56, 56, 256
NCORES = 8
P = 128
RC = (B // NCORES) * H * W  # rows per core = 12544
COLS = RC // P  # 98 row-slots per partition (= gather instructions)
T = 7  # row-slots per partition per super-tile
S = COLS // T  # 14 super-tiles
NPAD = RC + 2  # x_pad rows (zero row before and after)
C2 = 2 * C  # gathered chunk elems per row


def _setup(tc, cpool, offs_ap, t16_ap):
    """Constant tiles: j_iota (0..255 int16), offsets (u32), 256-m (i16)."""
    import concourse.mybir as mybir

    nc = tc.nc
    j_iota = cpool.tile([P, C], mybir.dt.int16)
    nc.gpsimd.iota(j_iota[:], pattern=[[1, C]], base=0, channel_multiplier=0)
    offs = cpool.tile([P, COLS], mybir.dt.uint32)
    nc.sync.dma_start(out=offs[:], in_=offs_ap)
    t16 = cpool.tile([P, COLS], mybir.dt.int16)
    nc.sync.dma_start(out=t16[:], in_=t16_ap)
    return {"j_iota": j_iota, "offs": offs, "t16": t16}


def _super_tile(tc, pool, consts, out_v, xflat_ap, u):
    """Gather T columns of 512-f32 chunks, stitch in place, store."""
    import concourse.mybir as mybir
    from concourse import bass

    nc = tc.nc
    csl = slice(u * T, (u + 1) * T)

    g = pool.tile([P, T, C2], mybir.dt.float32)
    mfull = pool.tile([P, T, C2], mybir.dt.uint8)
    mask = mfull[:, :, 0:C]
    for t in range(T):
        col = u * T + t
        nc.gpsimd.indirect_dma_start(
            out=g[:, t, :],
            out_offset=None,
            in_=xflat_ap,
            in_offset=bass.IndirectOffsetOnAxis(
                ap=consts["offs"][:, col : col + 1], axis=1
            ),
            element_offset=0,
        )
    # mask[p,t,j] = (j < 256 - m) -> overlay the head x[r, m:][j]
    nc.vector.tensor_tensor(
        out=mask,
        in0=consts["j_iota"][:].unsqueeze(1).to_broadcast([P, T, C]),
        in1=consts["t16"][:, csl].to_broadcast([P, T, C]),
        op=mybir.AluOpType.is_lt,
    )
    # chunk[j] <- chunk[256+j] where mask (disjoint halves, no hazard).
    nc.vector.copy_predicated(
        out=g[:, :, 0:C], mask=mask, data=g[:, :, C:C2]
    )
    nc.sync.dma_start(out=out_v[:, u], in_=g[:, :, 0:C])


def _build(tc, out_ap, xflat_ap, offs_ap, t16_ap):
    out_v = out_ap.rearrange("(s p t) c -> p s t c", s=S, p=P, t=T)
    with tc.tile_pool(name="const", bufs=1) as cpool:
        consts = _setup(tc, cpool, offs_ap, t16_ap)
        with tc.tile_pool(name="work", bufs=3) as pool:
            for u in range(S):
                _super_tile(tc, pool, consts, out_v, xflat_ap, u)


def _rowid_grid():
    """[P, COLS] with rid[p, s*T+t] = s*(P*T) + p*T + t (store layout)."""
    rid = (
        np.arange(S)[None, :, None] * (P * T)
        + np.arange(P)[:, None, None] * T
        + np.arange(T)[None, None, :]
    )
    return rid.reshape(P, COLS)


def _shard_inputs(x, shift_map):
    """Full inputs -> per-core {x_pad [1, NPAD*C] f32, offs [P,COLS] u32,
    t16 [P,COLS] i16}."""
    x = np.ascontiguousarray(np.asarray(x), dtype=np.float32)
    sm = np.asarray(shift_map).astype(np.int64)
    bpc = B // NCORES
    rid = _rowid_grid()
    in_maps = []
    for k in range(NCORES):
        xk = x[k * bpc : (k + 1) * bpc].reshape(RC, C)
        mk = sm[k * bpc : (k + 1) * bpc].reshape(RC)
        xpad = np.zeros((NPAD, C), np.float32)
        xpad[1 : RC + 1] = xk
        m_pt = mk[rid]  # [P, COLS]
        offs = (C * rid + m_pt).astype(np.uint32)
        t16 = (C - m_pt).astype(np.int16)
        in_maps.append(
            {"x_pad": xpad.reshape(1, -1), "offs": offs, "t16": t16}
        )
    return in_maps


_CACHE = {}


def _get_nc():
    key = "nc"
    if key in _CACHE:
        return _CACHE[key]
    import concourse.mybir as mybir
    import concourse.tile as tile
    from concourse import bacc

    nc = bacc.Bacc(
        "TRN2",
        debug=False,
        enable_asserts=False,
        num_devices=NCORES,
    )
    x_d = nc.dram_tensor(
        "x_pad", [1, NPAD * C], mybir.dt.float32, kind="ExternalInput"
    )
    o_d = nc.dram_tensor("offs", [P, COLS], mybir.dt.uint32, kind="ExternalInput")
    t_d = nc.dram_tensor("t16", [P, COLS], mybir.dt.int16, kind="ExternalInput")
    out_d = nc.dram_tensor("out", [RC, C], mybir.dt.float32, kind="ExternalOutput")
    with tile.TileContext(nc) as tc:
        _build(tc, out_d.ap(), x_d.ap(), o_d.ap(), t_d.ap())
    nc.compile()
    _CACHE[key] = nc
    return nc


def kernel(x, shift_map, trace=False):
    from concourse.bass_utils import run_bass_kernel_spmd

    nc = _get_nc()
    in_maps = _shard_inputs(x, shift_map)
    res = run_bass_kernel_spmd(
        nc, in_maps, core_ids=list(range(NCORES)), trace=trace
    )
    bpc = B // NCORES
    out = np.concatenate(
        [r["out"].reshape(bpc, H, W, C) for r in res.results], axis=0
    )
    if trace:
        kernel.last_results = res
    return out
